# revision 46
# baseline (speedup 1.0000x reference)
"""Trainium2 Bass kernel for a pre-norm transformer encoder layer with RoPE,
causal attention and SwiGLU FFN.

Sharding: data-parallel over batch (B=8 -> 8 NeuronCores, one batch element
per core).  Each core runs the full layer on its [S=1300, D=1080] slice.

Per-core dataflow (bf16 weights/activations, f32 residual/psum/LN stats;
W2 optionally fp8e4m3 DoubleRow):
  P1+P2 fused per token tile: LN1 on token-major x (r1), PE-transpose ->
      actT bf16 [128, 9, S]; V = x2 @ Wv token-major -> V_sb
      [128, 11, 12, 90] bf16, SBUF-resident
  P3  heads in groups of 2: (a) Q/K proj (M=90) + RoPE (rotation matmul),
      (b) per head: scoresT = K.Q^T per 512-wide q-chunk with causally
      trimmed columns, E = exp bf16, diag-tile zero-mask, attnT and the
      softmax denominator accumulated over key tiles, reciprocal + PE
      broadcast, normalized attnT SBUF-resident [90, 12, S] bf16
  P4+P5 fused per token tile: out-proj token-major (stationary attnT
      slices, moving Wo rows -> psum [sw, 360] accumulated over heads;
      r1 += psum, bo pre-added), then LN2 for the tile -> actT bf16
  P6  FFN: H = silu(x2'@W1)*(x2'@W3), H stored fp8 [128, 13, 2, 1312];
      W2 fp8 DoubleRow (weights x16, undone before the residual add);
      out = psum/16 + r1 (+b2 pre-added), one DMA per row tile.
      FP8W2=0 falls back to an all-bf16 FFN (rel err 1.5e-3 vs 1.1e-2).

Dependent DMAs (x, const blobs, outputs) issue from the gpsimd queue so
the SP queue stays a pure weight-prefetch stream across loop iterations.

The benchmark build (loop_n > 1) wraps the whole body in a device-side
For_i so one dispatch runs loop_n full layer passes back-to-back.
"""

import os
import sys

sys.path.insert(0, "/opt/trn_rl_repo")

import math

import numpy as np

B, S, D, H, DK, FF = 8, 1300, 1080, 12, 90, 3240
EPS = 1e-5

N_ST = (S + 127) // 128                      # 11 token tiles
SW = [128] * (N_ST - 1) + [S - 128 * (N_ST - 1)]   # last = 20
N_KT = (D + 127) // 128                      # 9
KP = [128] * (N_KT - 1) + [D - 128 * (N_KT - 1)]   # last = 56
QCH = [(0, 512), (512, 512), (1024, 276)]    # q/S chunks (128-aligned starts)
N_FT = (FF + 127) // 128                     # 26
FSZ = [128] * (N_FT - 1) + [FF - 128 * (N_FT - 1)]  # last = 40
N_VB = 3
VBW = D // N_VB                              # 360
W2VB = [(0, 384), (384, 352), (736, 344)]    # W2 output col chunks
NG = 5                                       # fp8 DoubleRow K-groups (5*256 >= D)
NTP = N_FT // 2                              # 13 fp8 ft-pairs for W2/H
SP8 = 1312                                   # S padded to %16 for fp8 plane stride
WS = 16.0                                    # fp8 W1/W3 pre-scale
HG = 2                                       # P3 head-group size
FP8_W2 = os.environ.get("FP8W2", "1") == "1"  # fp8 DoubleRow W2 (bf16 W1/W3)

# bf16 const blob column offsets
CB_ID = 0
CB_COS = 128
CB_SIN = CB_COS + S
CB_CM = CB_SIN + S
CB_RL = CB_CM + 4 * 512
NCB = CB_RL + DK
# f32 const blob column offsets
CF_BV = 0
CF_BO = D
CF_B2 = 2 * D
CF_G1 = 3 * D
CF_BE1 = CF_G1 + N_KT
CF_G2 = CF_BE1 + N_KT
CF_BE2 = CF_G2 + N_KT
CF_B1 = CF_BE2 + N_KT
CF_B3 = CF_B1 + N_FT
CF_BQ = CF_B3 + N_FT
CF_BK = CF_BQ + H
NCF = CF_BK + H

_CACHE = {}
PE_MARKS = []  # (phase_label, cumulative PE-instruction count) — sim profiling


def _build(loop_n=1, fp8=None):
    from contextlib import ExitStack

    import concourse.bacc as bacc
    import concourse.mybir as mybir
    import concourse.tile as tile

    if fp8 is None:
        fp8 = FP8_W2

    f32 = mybir.dt.float32
    f32r = mybir.dt.float32r
    bf16 = mybir.dt.bfloat16
    fp8e4 = mybir.dt.float8e4
    AF = mybir.ActivationFunctionType
    OP = mybir.AluOpType
    DR = mybir.MatmulPerfMode.DoubleRow

    nc = bacc.Bacc("TRN2", target_bir_lowering=False, debug=False)

    PE_MARKS.clear()
    _pe_n = [0]
    _mm = nc.tensor.matmul

    def _mm_w(*a, **k):
        _pe_n[0] += 1
        return _mm(*a, **k)

    nc.tensor.matmul = _mm_w

    def mark(label):
        PE_MARKS.append((label, _pe_n[0]))

    def din(name, shape, dt=f32):
        return nc.dram_tensor(name, shape, dt, kind="ExternalInput").ap()

    x_d = din("x", (S, D))
    cb_d = din("cb", (128, NCB), bf16)
    cf_d = din("cf", (128, NCF), f32)
    wqk_d = din("wqkr", (H, 128, 2, N_KT, DK), bf16)
    wv_d = din("wvr", (128, N_KT, D), bf16)
    wo_d = din("wor", (DK, H, D), bf16)
    w13_d = din("w13r", (N_FT, 128, 2, N_KT, 128), bf16)
    if fp8:
        w2_d = din("w2r8", (128, NTP, 2, 1088), fp8e4)
    else:
        w2_d = din("w2r", (128, N_FT, D), bf16)

    out_d = nc.dram_tensor("out", (S, D), f32, kind="ExternalOutput").ap()

    SCALE = 1.0 / math.sqrt(DK)

    with tile.TileContext(nc) as tc, ExitStack() as ctx:
        if loop_n > 1:
            # benchmark build: run the whole layer loop_n times per execute
            # so fixed per-dispatch overhead amortizes out of the timing.
            _hint = tuple(
                getattr(mybir.EngineType, e)
                for e in os.environ.get("LOOP_HINT", "").split(",") if e
            )
            ctx.enter_context(tc.For_i(
                0, loop_n, 1,
                hint_engines=_hint,
                staggered_reset=os.environ.get("LOOP_SR", "0") == "1",
            ))
        glob = ctx.enter_context(tc.tile_pool(name="glob", bufs=1))
        work = ctx.enter_context(tc.tile_pool(name="work", bufs=3))
        psA = ctx.enter_context(tc.tile_pool(name="psA", bufs=3, space="PSUM"))
        psB = ctx.enter_context(tc.tile_pool(name="psB", bufs=2, space="PSUM"))
        psC = ctx.enter_context(tc.tile_pool(name="psC", bufs=1, space="PSUM"))
        psT = ctx.enter_context(tc.tile_pool(name="psT", bufs=2, space="PSUM"))

        # ---------- persistent tensors (LIFO pool scoping) ----------
        r1 = glob.tile([128, N_ST, D], f32, tag="r1")
        pActT_cm = tc.tile_pool(name="pActT", bufs=1)
        pActT = pActT_cm.__enter__()
        actT = pActT.tile([128, N_KT, S], bf16, tag="actT")
        pAtt_cm = tc.tile_pool(name="pAtt", bufs=1)
        pAtt = pAtt_cm.__enter__()
        attnT = pAtt.tile([DK, H, S], bf16, tag="attnT")

        # ---------- first x tile, then constants (2 blob DMAs) ----------
        # x rows 0..511 load first so the LN1 chain starts while the const
        # blobs stream in behind it on the same gpsimd DMA queue.
        nc.gpsimd.dma_start(
            r1[:, :4, :], x_d[0:512].rearrange("(o p) d -> p o d", p=128)
        )
        cb = glob.tile([128, NCB], bf16, tag="cb")
        nc.gpsimd.dma_start(cb, cb_d)
        cf = glob.tile([128, NCF], f32, tag="cf")
        nc.gpsimd.dma_start(cf, cf_d)
        ident = cb[:, CB_ID : CB_ID + 128]
        cosT = cb[:DK, CB_COS : CB_COS + S]
        sinT = cb[:DK, CB_SIN : CB_SIN + S]
        rl_s = cb[:DK, CB_RL : CB_RL + DK]

        def cmv(t):
            return cb[:, CB_CM + 512 * t : CB_CM + 512 * (t + 1)]

        bv_bc = cf[:, CF_BV : CF_BV + D]
        bo_bc = cf[:, CF_BO : CF_BO + D]
        b2_bc = cf[:, CF_B2 : CF_B2 + D]
        g1_s = cf[:, CF_G1 : CF_G1 + N_KT]
        be1_s = cf[:, CF_BE1 : CF_BE1 + N_KT]
        g2_s = cf[:, CF_G2 : CF_G2 + N_KT]
        be2_s = cf[:, CF_BE2 : CF_BE2 + N_KT]
        b1_s = cf[:, CF_B1 : CF_B1 + N_FT]
        b3_s = cf[:, CF_B3 : CF_B3 + N_FT]
        bq_s = cf[:DK, CF_BQ : CF_BQ + H]
        bk_s = cf[:DK, CF_BK : CF_BK + H]

        ones_row = glob.tile([1, 128], f32r, tag="onesrow")
        nc.vector.memset(ones_row.bitcast(f32), 1.0)
        ones_col = glob.tile([128, 1], bf16, tag="onescol")
        nc.vector.memset(ones_col, 1.0)
        eps_t = glob.tile([128, 1], f32, tag="eps")
        nc.vector.memset(eps_t, EPS)

        # ---------- helper: LN + transpose for one token tile ----------
        def ln_tile(st, g_s, be_s, write):
            sw = SW[st]
            xt = r1[:sw, st, :]
            ssum = work.tile([128, 1], f32, tag="ssum")
            nc.vector.reduce_sum(ssum[:sw], xt, axis=mybir.AxisListType.X)
            sqd = work.tile([128, D], f32, tag="sqdump")
            ssq = work.tile([128, 1], f32, tag="ssq")
            nc.scalar.activation(sqd[:sw], xt, AF.Square, accum_out=ssq[:sw])
            mean = work.tile([128, 1], f32, tag="mean")
            nc.scalar.mul(mean[:sw], ssum[:sw], 1.0 / D)
            msq = work.tile([128, 1], f32, tag="msq")
            nc.vector.tensor_mul(msq[:sw], mean[:sw], mean[:sw])
            var = work.tile([128, 1], f32, tag="var")
            nc.vector.tensor_scalar_mul(var[:sw], ssq[:sw], 1.0 / D)
            nc.vector.tensor_sub(var[:sw], var[:sw], msq[:sw])
            std = work.tile([128, 1], f32, tag="std")
            nc.scalar.activation(std[:sw], var[:sw], AF.Sqrt, bias=eps_t[:sw])
            rstd = work.tile([128, 1], f32, tag="rstd")
            nc.vector.reciprocal(rstd[:sw], std[:sw])
            xn = work.tile([128, D], bf16, tag="xn")
            nc.vector.tensor_scalar(
                xn[:sw], xt, scalar1=mean[:sw], scalar2=rstd[:sw],
                op0=OP.subtract, op1=OP.mult,
            )
            for kt in range(N_KT):
                kp = KP[kt]
                d0 = 128 * kt
                pt = psT.tile([128, 128], bf16, tag="pst")
                nc.tensor.transpose(
                    pt[:kp, :sw], xn[:sw, d0 : d0 + kp], ident[:sw, :sw]
                )
                write(kt, st, sw, pt, g_s, be_s)

        def write_actT(kt, st, sw, pt, g_s, be_s):
            kp = KP[kt]
            nc.scalar.activation(
                actT[:kp, kt, 128 * st : 128 * st + sw], pt[:kp, :sw],
                AF.Identity,
                bias=be_s[:kp, kt : kt + 1], scale=g_s[:kp, kt : kt + 1],
            )

        # ========= P1+P2 fused per tile: load x, LN1, V proj =========
        # x / const loads go through the gpsimd DMA queue so the SP queue
        # stays a pure weight-prefetch stream (no head-of-line blocking on
        # the previous loop iteration's compute).
        nc.gpsimd.dma_start(
            r1[:, 4:8, :], x_d[512:1024].rearrange("(o p) d -> p o d", p=128)
        )
        nc.gpsimd.dma_start(
            r1[:, 8:10, :], x_d[1024:1280].rearrange("(o p) d -> p o d", p=128)
        )
        nc.gpsimd.dma_start(r1[:20, 10, :], x_d[1280:1300])
        pV_cm = tc.tile_pool(name="pV", bufs=1)
        pV = pV_cm.__enter__()
        V_sb = pV.tile([128, N_ST, H, DK], bf16, tag="V")
        with tc.tile_pool(name="pP2w", bufs=1) as pP2w:
            wv_t = pP2w.tile([128, N_KT, D], bf16, tag="wv")
            nc.sync.dma_start(wv_t, wv_d)
            for st in range(N_ST):
                sw = SW[st]
                s0 = 128 * st
                ln_tile(st, g1_s, be1_s, write_actT)
                for vb in range(N_VB):
                    c0 = VBW * vb
                    pv = psA.tile([128, VBW], f32, tag="pa")
                    for kt in range(N_KT):
                        kp = KP[kt]
                        nc.tensor.matmul(
                            pv[:sw],
                            actT[:kp, kt, s0 : s0 + sw],
                            wv_t[:kp, kt, c0 : c0 + VBW],
                            start=(kt == 0),
                            stop=(kt == N_KT - 1),
                        )
                    nc.vector.tensor_tensor(
                        V_sb[:sw, st, 4 * vb : 4 * vb + 4, 0:DK],
                        pv[:sw], bv_bc[:sw, c0 : c0 + VBW], OP.add,
                    )
        # r1 += bo (gpsimd; overlaps attention)
        for st in range(N_ST):
            nc.gpsimd.tensor_tensor(
                r1[: SW[st], st, :], r1[: SW[st], st, :], bo_bc[: SW[st]], OP.add
            )
        mark("P2_v")

        # ================= P3: attention (head groups) =================
        with tc.tile_pool(name="pP3w", bufs=2) as pP3w, \
             tc.tile_pool(name="pQK", bufs=1) as pQK, \
             tc.tile_pool(name="pP3", bufs=2) as pP3, \
             tc.tile_pool(name="pRec", bufs=1) as pRec, \
             tc.tile_pool(name="pP3e", bufs=6) as pP3e:
            for h0 in range(0, H, HG):
                # --- (a) Q/K proj + RoPE for the group ---
                qkT = pQK.tile([DK, HG, 2, S], bf16, tag="qkT")
                for hh in range(HG):
                    h = h0 + hh
                    wqk_t = pP3w.tile([128, 2, N_KT, DK], bf16, tag="wqk")
                    nc.sync.dma_start(wqk_t, wqk_d[h])
                    for j, b_s in ((0, bq_s), (1, bk_s)):
                        for (q0, qw) in QCH:
                            pq = psT.tile([DK, 512], f32, tag="pst")
                            for kt in range(N_KT):
                                kp = KP[kt]
                                nc.tensor.matmul(
                                    pq[:, :qw],
                                    wqk_t[:kp, j, kt, :],
                                    actT[:kp, kt, q0 : q0 + qw],
                                    start=(kt == 0),
                                    stop=(kt == N_KT - 1),
                                )
                            raw = pP3.tile([DK, 512], bf16, tag="qraw")
                            nc.scalar.activation(
                                raw[:, :qw], pq[:, :qw], AF.Identity,
                                bias=b_s[:, h : h + 1],
                            )
                            prot = psT.tile([DK, 512], f32, tag="pst")
                            nc.tensor.matmul(
                                prot[:, :qw], rl_s, raw[:, :qw],
                                start=True, stop=True,
                            )
                            t1 = pP3.tile([DK, 512], bf16, tag="ropet1")
                            nc.gpsimd.tensor_tensor(
                                t1[:, :qw], raw[:, :qw],
                                cosT[:, q0 : q0 + qw], OP.mult,
                            )
                            t2 = pP3.tile([DK, 512], bf16, tag="ropet2")
                            nc.vector.tensor_tensor(
                                t2[:, :qw], prot[:, :qw],
                                sinT[:, q0 : q0 + qw], OP.mult,
                            )
                            nc.vector.tensor_tensor(
                                qkT[:, hh, j, q0 : q0 + qw], t1[:, :qw],
                                t2[:, :qw], OP.add,
                            )
                # --- (b) attention for each head in the group ---
                for hh in range(HG):
                    h = h0 + hh
                    for (q0, qw) in QCH:
                        kmax = min(N_ST, (q0 + qw + 127) // 128)
                        pat = psB.tile([DK, 512], f32, tag="pb")
                        pden = psC.tile([1, 512], f32, tag="pc")
                        for i in range(kmax):
                            ksz = SW[i]
                            t_ = i - q0 // 128
                            c0 = max(0, 128 * t_)
                            cw = qw - c0
                            pe = psA.tile([128, 512], f32, tag="pa")
                            nc.tensor.matmul(
                                pe[:ksz, :cw],
                                qkT[:, hh, 1, 128 * i : 128 * i + ksz],
                                qkT[:, hh, 0, q0 + c0 : q0 + qw],
                                start=True,
                                stop=True,
                            )
                            et = pP3e.tile([128, 512], bf16, tag="et")
                            nc.scalar.activation(
                                et[:ksz, :cw], pe[:ksz, :cw], AF.Exp,
                                scale=SCALE,
                            )
                            if t_ >= 0:
                                nc.vector.tensor_tensor(
                                    et[:ksz, :cw], et[:ksz, :cw],
                                    cmv(t_)[:ksz, c0 : c0 + cw], OP.mult,
                                )
                            nc.tensor.matmul(
                                pat[:, c0:qw], V_sb[:ksz, i, h, :],
                                et[:ksz, :cw],
                                start=(i == 0), stop=(i == kmax - 1),
                            )
                            nc.tensor.matmul(
                                pden[:, c0:qw], ones_col[:ksz],
                                et[:ksz, :cw],
                                start=(i == 0), stop=(i == kmax - 1),
                            )
                        rec = pRec.tile([1, 512], f32r, tag="rec")
                        with nc.allow_low_precision(reason="f32r denom"):
                            nc.vector.reciprocal(rec[:, :qw], pden[:, :qw])
                        bcp = psC.tile([DK, 512], f32, tag="pc")
                        nc.tensor.matmul(
                            bcp[:, :qw], ones_row[:1, :DK], rec[:, :qw],
                            start=True, stop=True,
                        )
                        bc = pP3.tile([DK, 512], bf16, tag="bc")
                        nc.vector.tensor_copy(bc[:, :qw], bcp[:, :qw])
                        nc.vector.tensor_tensor(
                            attnT[:, h, q0 : q0 + qw], pat[:DK, :qw],
                            bc[:, :qw], OP.mult,
                        )
        pV_cm.__exit__(None, None, None)
        mark("P3_attn")

        # ================= P4 + P5: out-proj + residual + LN2 ===============
        with tc.tile_pool(name="pP4w", bufs=1) as pP4w:
            wo_t = pP4w.tile([DK, H, D], bf16, tag="wo")
            nc.sync.dma_start(wo_t, wo_d)
            for st in range(N_ST):
                sw = SW[st]
                s0 = 128 * st
                for vb in range(N_VB):
                    c0 = VBW * vb
                    po = psA.tile([128, VBW], f32, tag="pa")
                    for hh in range(H):
                        nc.tensor.matmul(
                            po[:sw],
                            attnT[:, hh, s0 : s0 + sw],
                            wo_t[:, hh, c0 : c0 + VBW],
                            start=(hh == 0), stop=(hh == H - 1),
                        )
                    nc.vector.tensor_tensor(
                        r1[:sw, st, c0 : c0 + VBW], r1[:sw, st, c0 : c0 + VBW],
                        po[:sw], OP.add,
                    )
                # LN2 for this tile pipelines behind the next tile's out-proj
                ln_tile(st, g2_s, be2_s, write_actT)
        pAtt_cm.__exit__(None, None, None)
        # r1 += b2 (overlaps FFN)
        for st in range(N_ST):
            nc.gpsimd.tensor_tensor(
                r1[: SW[st], st, :], r1[: SW[st], st, :], b2_bc[: SW[st]], OP.add
            )
        mark("P5_ln2")

        # ================= P6: FFN =================
        with tc.tile_pool(name="pHt", bufs=1) as pHt, \
             tc.tile_pool(name="pFw", bufs=2) as pFw, \
             tc.tile_pool(name="pFw2", bufs=1) as pFw2, \
             tc.tile_pool(name="pFo", bufs=2 if fp8 else 1) as pFo:
            if fp8:
                Ht8 = pHt.tile([128, NTP, 2, SP8], fp8e4, tag="Ht8")
                # pre-zero the partially-filled last ft plane (NaN-safe DR);
                # the W13 writes fill the valid rows afterwards
                nc.vector.memset(Ht8[:, NTP - 1, 1, :], 0.0)
            else:
                Ht = pHt.tile([128, N_FT, S], bf16, tag="Ht")
            for ft in range(N_FT):
                fsz = FSZ[ft]
                w13_t = pFw.tile([128, 2, N_KT, 128], bf16, tag="w13")
                nc.sync.dma_start(w13_t, w13_d[ft])
                for (q0, qw) in QCH:
                    p1_ = psA.tile([128, 512], f32, tag="pa")
                    p3_ = psB.tile([128, 512], f32, tag="pb")
                    for kt in range(N_KT):
                        kp = KP[kt]
                        nc.tensor.matmul(
                            p1_[:fsz, :qw], w13_t[:kp, 0, kt, :fsz],
                            actT[:kp, kt, q0 : q0 + qw],
                            start=(kt == 0), stop=(kt == N_KT - 1),
                        )
                        nc.tensor.matmul(
                            p3_[:fsz, :qw], w13_t[:kp, 1, kt, :fsz],
                            actT[:kp, kt, q0 : q0 + qw],
                            start=(kt == 0), stop=(kt == N_KT - 1),
                        )
                    h1s = pFw.tile([128, 512], bf16, tag="h1s")
                    nc.scalar.activation(
                        h1s[:fsz, :qw], p1_[:fsz, :qw], AF.Silu,
                        bias=b1_s[:fsz, ft : ft + 1],
                    )
                    h3b = pFw.tile([128, 512], bf16, tag="h3b")
                    nc.scalar.activation(
                        h3b[:fsz, :qw], p3_[:fsz, :qw], AF.Identity,
                        bias=b3_s[:fsz, ft : ft + 1],
                    )
                    hdst = (Ht8[:fsz, ft // 2, ft % 2, q0 : q0 + qw]
                            if fp8 else Ht[:fsz, ft, q0 : q0 + qw])
                    nc.vector.tensor_tensor(
                        hdst, h1s[:fsz, :qw], h3b[:fsz, :qw], OP.mult,
                    )
            mark("P6_w13")
            if fp8:
                # W2 fp8 DoubleRow; weights pre-scaled x16 (undone below)
                w2_t = pFw2.tile([128, NTP, 2, 1088], fp8e4, tag="w2")
                nc.sync.dma_start(w2_t, w2_d)
                for st in range(N_ST):
                    sw = SW[st]
                    s0 = 128 * st
                    osb = pFo.tile([128, D], f32, tag="osb")
                    for (c0, cw) in W2VB:
                        pf = psA.tile([128, 512], f32, tag="pa")
                        for tp in range(NTP):
                            nc.tensor.matmul(
                                pf[:sw, :cw], Ht8[:, tp, :, s0 : s0 + sw],
                                w2_t[:, tp, :, c0 : c0 + cw],
                                start=(tp == 0), stop=(tp == NTP - 1),
                                perf_mode=DR,
                            )
                        fsc = pFo.tile([128, 512], f32, tag="fsc")
                        nc.vector.tensor_scalar_mul(
                            fsc[:sw, :cw], pf[:sw, :cw], 1.0 / WS
                        )
                        nc.gpsimd.tensor_tensor(
                            osb[:sw, c0 : c0 + cw], fsc[:sw, :cw],
                            r1[:sw, st, c0 : c0 + cw], OP.add,
                        )
                    nc.gpsimd.dma_start(out_d[s0 : s0 + sw, :], osb[:sw])
            else:
                for (c0, cw) in W2VB:
                    w2_t = pFw2.tile([128, N_FT, 384], bf16, tag="w2")
                    nc.sync.dma_start(w2_t[:, :, :cw], w2_d[:, :, c0 : c0 + cw])
                    for st in range(N_ST):
                        sw = SW[st]
                        s0 = 128 * st
                        pf = psA.tile([128, 512], f32, tag="pa")
                        for ft in range(N_FT):
                            fsz = FSZ[ft]
                            nc.tensor.matmul(
                                pf[:sw, :cw], Ht[:fsz, ft, s0 : s0 + sw],
                                w2_t[:fsz, ft, :cw],
                                start=(ft == 0), stop=(ft == N_FT - 1),
                            )
                        osb = pFo.tile([128, 512], f32, tag="osb")
                        nc.vector.tensor_tensor(
                            osb[:sw, :cw], pf[:sw, :cw],
                            r1[:sw, st, c0 : c0 + cw], OP.add,
                        )
                        nc.gpsimd.dma_start(
                            out_d[s0 : s0 + sw, c0 : c0 + cw], osb[:sw, :cw]
                        )
            mark("P6_w2")
        pActT_cm.__exit__(None, None, None)

    nc.compile()
    return nc


def _host_inputs(inputs):
    """Shared (per-core-identical) input map pieces, from full inputs."""
    import ml_dtypes

    bf = ml_dtypes.bfloat16
    f8 = ml_dtypes.float8_e4m3fn
    f = lambda k: np.ascontiguousarray(np.asarray(inputs[k], np.float32))

    def pad(w, rows, cols):
        out = np.zeros((rows, cols), np.float32)
        out[: w.shape[0], : w.shape[1]] = w
        return out

    # bf16 const blob
    cbb = np.zeros((128, NCB), np.float32)
    cbb[:, CB_ID : CB_ID + 128] = np.eye(128, dtype=np.float32)
    cbb[:DK, CB_COS : CB_COS + S] = f("rope_cos").T
    cbb[:DK, CB_SIN : CB_SIN + S] = f("rope_sin").T
    p_, f_ = np.mgrid[0:128, 0:512]
    for t in range(4):
        cbb[:, CB_CM + 512 * t : CB_CM + 512 * (t + 1)] = (
            f_ >= p_ + 128 * t
        ).astype(np.float32)
    hdk = DK // 2
    rl = np.zeros((DK, DK), np.float32)
    rl[np.arange(hdk) + hdk, np.arange(hdk)] = -1.0
    rl[np.arange(hdk), np.arange(hdk) + hdk] = 1.0
    cbb[:DK, CB_RL : CB_RL + DK] = rl

    # f32 const blob
    cff = np.zeros((128, NCF), np.float32)
    cff[:, CF_BV : CF_BV + D] = np.broadcast_to(f("bv")[None, :], (128, D))
    cff[:, CF_BO : CF_BO + D] = np.broadcast_to(f("bo")[None, :], (128, D))
    cff[:, CF_B2 : CF_B2 + D] = np.broadcast_to(f("b2")[None, :], (128, D))

    def col(dst_off, vec, n, psz):
        v = np.asarray(vec, np.float32)
        for i in range(n):
            o = sum(psz[:i])
            cff[: psz[i], dst_off + i] = v[o : o + psz[i]]

    col(CF_G1, f("ln1_g"), N_KT, KP)
    col(CF_BE1, f("ln1_b"), N_KT, KP)
    col(CF_G2, f("ln2_g"), N_KT, KP)
    col(CF_BE2, f("ln2_b"), N_KT, KP)
    col(CF_B1, f("b1"), N_FT, FSZ)
    col(CF_B3, f("b3"), N_FT, FSZ)
    col(CF_BQ, f("bq"), H, [DK] * H)
    col(CF_BK, f("bk"), H, [DK] * H)

    Wq = f("Wq"); Wk = f("Wk"); Wv = f("Wv"); Wo = f("Wo")
    W1 = f("W1"); W3 = f("W3"); W2 = f("W2")
    KR = N_KT * 128
    FR = N_FT * 128
    # [H, 128, 2, N_KT, DK]: (h, p, j, o, d) = W{q,k}[o*128+p, h*90+d]
    wq4 = pad(Wq, KR, D).reshape(N_KT, 128, H, DK)
    wk4 = pad(Wk, KR, D).reshape(N_KT, 128, H, DK)
    # stack axis0=j -> [2, N_KT, 128, H, DK]; transpose to (h, p, j, o, d)
    wqk = np.stack([wq4, wk4], axis=0).transpose(3, 2, 0, 1, 4)
    wvr = pad(Wv, KR, D).reshape(N_KT, 128, D).transpose(1, 0, 2)
    wor = Wo.reshape(H, DK, D).transpose(1, 0, 2)

    out = {
        "cb": np.ascontiguousarray(cbb).astype(bf),
        "cf": np.ascontiguousarray(cff),
        "wqkr": np.ascontiguousarray(wqk).astype(bf),
        "wvr": np.ascontiguousarray(wvr).astype(bf),
        "wor": np.ascontiguousarray(wor).astype(bf),
    }
    # [N_FT, 128, 2, N_KT, 128]: (o, p, j, k, m) = W{1,3}[128k+p, 128o+m]
    w14 = pad(W1, KR, FR).reshape(N_KT, 128, N_FT, 128)
    w34 = pad(W3, KR, FR).reshape(N_KT, 128, N_FT, 128)
    out["w13r"] = np.ascontiguousarray(
        np.stack([w14, w34], axis=0).transpose(3, 2, 0, 1, 4)).astype(bf)
    if FP8_W2:
        # [128, NTP, 2, 1088]: (p, tp, i, m) = WS*W2[256tp+128i+p, m]
        w2p = pad(W2 * WS, NTP * 256, 1088).reshape(NTP, 2, 128, 1088)
        out["w2r8"] = np.ascontiguousarray(w2p.transpose(2, 0, 1, 3)).astype(f8)
    else:
        out["w2r"] = np.ascontiguousarray(
            pad(W2, FR, D).reshape(N_FT, 128, D).transpose(1, 0, 2)).astype(bf)
    return out


def kernel(**inputs):
    from concourse.bass_utils import run_bass_kernel_spmd

    if "nc" not in _CACHE:
        _CACHE["nc"] = _build()
    nc = _CACHE["nc"]

    shared = _host_inputs(inputs)
    x = np.asarray(inputs["x"], np.float32)
    in_maps = [dict(shared, x=np.ascontiguousarray(x[b])) for b in range(B)]
    res = run_bass_kernel_spmd(nc, in_maps, list(range(B))).results
    out = np.stack([res[b]["out"] for b in range(B)], axis=0)
    return out.astype(np.float32)
